# revision 1
# baseline (speedup 1.0000x reference)
"""MoE routing kernel for Trainium2 (8 NeuronCores, expert-parallel).

Strategy:
  - Host: compute gate (sigmoid + grouped top-k routing) in numpy, gather
    tokens per expert (sparse dispatch; top-2 of 8 experts per token).
  - Device (SPMD, core e): SwiGLU MLP with expert e's weights over the
    tokens routed to e, plus a 1/8 token-shard of the shared-expert MLP.
    Layout keeps features on SBUF partitions and streams tokens along the
    free axis, so activations feed matmuls without any on-device transpose.
    Matmuls run as float32r (full-rate single-pass, fp32 storage).
  - Host: weighted scatter-add of expert outputs + shared output.
"""

import numpy as np
from contextlib import ExitStack

DIM = 768
INTER = 512
E = 8
G = 4
TOPK = 2
N_CORES = 8
P = 128
NCHUNK = 512  # tokens per PSUM tile (fp32 bank limit)


# ---------------------------------------------------------------- host gate
def _host_gate(x2, gate_weight, gate_bias):
    """Reproduces reference._gate in numpy f32. Returns (w [T,2], idx [T,2])."""
    T = x2.shape[0]
    logits = x2 @ gate_weight.T
    scores = 1.0 / (1.0 + np.exp(-logits, dtype=np.float32))
    s = scores + gate_bias
    sv = s.reshape(T, G, E // G)
    group_scores = sv.sum(-1)  # top-2 of 2 per group == sum
    gidx = np.argsort(-group_scores, axis=1, kind="stable")[:, :2]
    gmask = np.zeros((T, G), bool)
    gmask[np.arange(T)[:, None], gidx] = True
    masked = np.where(gmask[:, :, None], sv, -np.inf).reshape(T, E)
    idx = np.argsort(-masked, axis=1, kind="stable")[:, :TOPK]
    w = np.take_along_axis(scores, idx, axis=1)
    w = w / (w.sum(-1, keepdims=True) + 1e-6)
    return w.astype(np.float32), idx.astype(np.int32)


# ---------------------------------------------------------- device kernel IR
def _build_nc(cap, nsh):
    import concourse.bass as bass
    import concourse.tile as tile
    from concourse import bacc, mybir

    f32 = mybir.dt.float32
    f32r = mybir.dt.float32r
    KD = DIM // P    # 6 k-tiles over model dim
    KI = INTER // P  # 4 k-tiles over inter dim

    nc = bacc.Bacc(
        "TRN2",
        target_bir_lowering=False,
        debug=False,
        enable_asserts=False,
        num_devices=N_CORES,
    )

    xg = nc.dram_tensor("xg", [DIM, cap], f32r, kind="ExternalInput").ap()
    xs = nc.dram_tensor("xs", [DIM, nsh], f32r, kind="ExternalInput").ap()
    w1t = nc.dram_tensor("w1t", [DIM, INTER], f32r, kind="ExternalInput").ap()
    w3t = nc.dram_tensor("w3t", [DIM, INTER], f32r, kind="ExternalInput").ap()
    w2t = nc.dram_tensor("w2t", [INTER, DIM], f32r, kind="ExternalInput").ap()
    sw1t = nc.dram_tensor("sw1t", [DIM, INTER], f32r, kind="ExternalInput").ap()
    sw3t = nc.dram_tensor("sw3t", [DIM, INTER], f32r, kind="ExternalInput").ap()
    sw2t = nc.dram_tensor("sw2t", [INTER, DIM], f32r, kind="ExternalInput").ap()
    oe = nc.dram_tensor("oe", [DIM, cap], f32, kind="ExternalOutput").ap()
    oz = nc.dram_tensor("oz", [DIM, nsh], f32, kind="ExternalOutput").ap()

    with tile.TileContext(nc) as tc, ExitStack() as ctx:
        wpool = ctx.enter_context(tc.tile_pool(name="wpool", bufs=1))
        xpool = ctx.enter_context(tc.tile_pool(name="xpool", bufs=3))
        hpool = ctx.enter_context(tc.tile_pool(name="hpool", bufs=2))
        sgpool = ctx.enter_context(tc.tile_pool(name="sgpool", bufs=2))
        opool = ctx.enter_context(tc.tile_pool(name="opool", bufs=2))
        # p1/p3 get 3 banks each, p2 gets 2: all 8 PSUM banks in play
        ppool = ctx.enter_context(tc.tile_pool(name="ppool", bufs=3, space="PSUM"))
        ppool2 = ctx.enter_context(tc.tile_pool(name="ppool2", bufs=2, space="PSUM"))

        xsr = xs.rearrange("(kt p) n -> p kt n", p=P)

        # Critical-path prologue on the fast sync queue: interleave the first
        # shared x-chunk with the first-needed weight (sw1) per k-tile, so the
        # first matmul starts after ~2 small transfers instead of ~3MB.
        n_first = min(NCHUNK, nsh)
        xt0 = xpool.tile([P, KD, NCHUNK], f32r, tag="xt", name="xt0")
        sw1s = wpool.tile([P, KD, INTER], f32r, tag="sw1s", name="sw1s")
        sw1r = sw1t.rearrange("(kt p) m -> p kt m", p=P)
        for k in range(KD):
            nc.sync.dma_start(out=xt0[:, k, :n_first], in_=xsr[:, k, :n_first])
            nc.sync.dma_start(out=sw1s[:, k, :], in_=sw1r[:, k, :])

        def load_weight(ap_, tag):
            # DRAM [K, M] -> SBUF [P, K//P, M]; lhsT slices are [:, k, m*P:(m+1)*P]
            # per-k-tile DMAs on the gpsimd queue (no competing work there)
            kt = ap_.shape[0] // P
            t = wpool.tile([P, kt, ap_.shape[1]], f32r, tag=tag, name=tag)
            src = ap_.rearrange("(kt p) m -> p kt m", p=P)
            for k in range(kt):
                nc.gpsimd.dma_start(out=t[:, k, :], in_=src[:, k, :])
            return t

        sw3s = load_weight(sw3t, "sw3s")
        sw2s = load_weight(sw2t, "sw2s")
        w1s = load_weight(w1t, "w1s")
        w3s = load_weight(w3t, "w3s")
        w2s = load_weight(w2t, "w2s")

        def swiglu(xT, outT, a1, a3, a2, ntok, xt_pre=None):
            xTr = xT.rearrange("(kt p) n -> p kt n", p=P)
            oTr = outT.rearrange("(kt p) n -> p kt n", p=P)
            nchunks = (ntok + NCHUNK - 1) // NCHUNK
            for c in range(nchunks):
                n0 = c * NCHUNK
                n = min(NCHUNK, ntok - n0)
                if c == 0 and xt_pre is not None:
                    xt = xt_pre
                else:
                    xt = xpool.tile([P, KD, NCHUNK], f32r, tag="xt", name="xt")
                    nc.sync.dma_start(out=xt[:, :, :n], in_=xTr[:, :, n0 : n0 + n])
                h = hpool.tile([P, KI, NCHUNK], f32r, tag="h", name="h")
                for m in range(KI):
                    p1 = ppool.tile([P, NCHUNK], f32, tag="p1", name="p1")
                    for k in range(KD):
                        nc.tensor.matmul(
                            p1[:, :n],
                            a1[:, k, m * P : (m + 1) * P],
                            xt[:, k, :n],
                            start=(k == 0),
                            stop=(k == KD - 1),
                        )
                    # silu(x) = x * sigmoid(x)
                    sg = sgpool.tile([P, NCHUNK], f32, tag="sg", name="sg")
                    nc.scalar.activation(
                        sg[:, :n], p1[:, :n], mybir.ActivationFunctionType.Sigmoid
                    )
                    nc.vector.tensor_mul(h[:, m, :n], sg[:, :n], p1[:, :n])
                    p3 = ppool.tile([P, NCHUNK], f32, tag="p3", name="p3")
                    for k in range(KD):
                        nc.tensor.matmul(
                            p3[:, :n],
                            a3[:, k, m * P : (m + 1) * P],
                            xt[:, k, :n],
                            start=(k == 0),
                            stop=(k == KD - 1),
                        )
                    nc.vector.tensor_mul(h[:, m, :n], h[:, m, :n], p3[:, :n])
                ot = opool.tile([P, KD, NCHUNK], f32, tag="ot", name="ot")
                for m2 in range(KD):
                    p2 = ppool2.tile([P, NCHUNK], f32, tag="p2", name="p2")
                    for k2 in range(KI):
                        nc.tensor.matmul(
                            p2[:, :n],
                            a2[:, k2, m2 * P : (m2 + 1) * P],
                            h[:, k2, :n],
                            start=(k2 == 0),
                            stop=(k2 == KI - 1),
                        )
                    nc.vector.tensor_copy(ot[:, m2, :n], p2[:, :n])
                nc.sync.dma_start(out=oTr[:, :, n0 : n0 + n], in_=ot[:, :, :n])

        # shared phase first: the routed remainder chunk (smallest) drains last
        swiglu(xs, oz, sw1s, sw3s, sw2s, nsh, xt_pre=xt0)
        swiglu(xg, oe, w1s, w3s, w2s, cap)

    nc.compile()
    return nc


# ------------------------------------------------------------------- driver
def kernel(x, gate_weight, gate_bias, w1, w2, w3, sw1, sw2, sw3):
    from concourse.bass_utils import run_bass_kernel_spmd

    B, S, D = x.shape
    x2 = np.ascontiguousarray(x.reshape(-1, D))
    T = x2.shape[0]
    nsh = T // N_CORES

    w, idx = _host_gate(x2, gate_weight, gate_bias)

    rows_per_e = [np.nonzero((idx == e).any(axis=1))[0] for e in range(E)]
    cap = max(len(r) for r in rows_per_e)
    cap = ((cap + P - 1) // P) * P

    nc = _build_nc(cap, nsh)

    x2T = np.ascontiguousarray(x2.T)  # [D, T]
    in_maps = []
    for e in range(E):
        rows = rows_per_e[e]
        xgT = np.zeros((DIM, cap), np.float32)
        xgT[:, : len(rows)] = x2T[:, rows]
        in_maps.append(
            {
                "xg": xgT,
                "xs": np.ascontiguousarray(x2T[:, e * nsh : (e + 1) * nsh]),
                "w1t": np.ascontiguousarray(w1[e].T),
                "w3t": np.ascontiguousarray(w3[e].T),
                "w2t": np.ascontiguousarray(w2[e].T),
                "sw1t": np.ascontiguousarray(sw1.T),
                "sw3t": np.ascontiguousarray(sw3.T),
                "sw2t": np.ascontiguousarray(sw2.T),
            }
        )

    r = run_bass_kernel_spmd(nc, in_maps, list(range(N_CORES)))
    globals()["LAST_RESULTS"] = r
    res = r.results

    y = np.zeros((T, D), np.float32)
    for e in range(E):
        rows = rows_per_e[e]
        cnt = len(rows)
        Oe = res[e]["oe"][:, :cnt].T  # [cnt, D]
        we = np.where(idx[rows, 0] == e, w[rows, 0], w[rows, 1]).astype(np.float32)
        y[rows] += we[:, None] * Oe
    z = np.concatenate([res[c]["oz"].T for c in range(N_CORES)], axis=0)  # [T, D]
    return (y + z).reshape(B, S, D)



# revision 5
# speedup vs baseline: 1.0648x; 1.0648x over previous
"""MoE routing kernel for Trainium2 (8 NeuronCores, expert-parallel).

Strategy (v2):
  - Host: gate (sigmoid + grouped top-k) in numpy; gather tokens per expert.
  - Device (SPMD, core e): SwiGLU MLP with expert e's weights over the tokens
    routed to e (padded to the max expert load), plus a 1/8 token shard of the
    shared-expert MLP.  All matmul operands are bf16 (fp32 PSUM accumulation):
    same PE rate as fp32r but half the DMA/SBUF traffic and fast weight loads.
  - Everything is host-pre-tiled into per-chunk [128, kt, n] blocks so each
    DMA descriptor moves long contiguous runs; loads/stores are split across
    several descriptors (and engine queues) so many DMA engines run in
    parallel — this was the v1 startup/tail bottleneck.
  - A short burst of dummy matmuls on a zeroed tile warms the PE clock (HAM)
    while the first real data is still in flight.
  - Host: weighted scatter-add of expert outputs + shared output.
"""

import numpy as np
import ml_dtypes
from contextlib import ExitStack

DIM = 768
INTER = 512
E = 8
G = 4
TOPK = 2
N_CORES = 8
P = 128
KD = DIM // P    # 6 k-tiles over model dim
KI = INTER // P  # 4 k-tiles over inter dim
NSH = 2048       # shared tokens per core
BF = ml_dtypes.bfloat16

SHARED_SIZES = [256, 512, 512, 512, 256]  # == NSH; small first chunk -> fast start
N_WARMUP = 11   # dummy matmuls to warm the PE clock during initial DMA


# ---------------------------------------------------------------- host gate
def _host_gate(x2, gate_weight, gate_bias):
    """Reproduces reference._gate in numpy f32. Returns (w [T,2], idx [T,2])."""
    T = x2.shape[0]
    logits = x2 @ gate_weight.T
    scores = 1.0 / (1.0 + np.exp(-logits, dtype=np.float32))
    s = scores + gate_bias
    sv = s.reshape(T, G, E // G)
    group_scores = sv.sum(-1)  # top-2 of 2 per group == sum
    gidx = np.argsort(-group_scores, axis=1, kind="stable")[:, :2]
    gmask = np.zeros((T, G), bool)
    gmask[np.arange(T)[:, None], gidx] = True
    masked = np.where(gmask[:, :, None], sv, -np.inf).reshape(T, E)
    idx = np.argsort(-masked, axis=1, kind="stable")[:, :TOPK]
    w = np.take_along_axis(scores, idx, axis=1)
    w = w / (w.sum(-1, keepdims=True) + 1e-6)
    return w.astype(np.float32), idx.astype(np.int32)


def _plan(total):
    """Split into <=512 chunks; keep every chunk >=256 tokens (LDW-bound tails
    are slow), sizes multiples of 32."""
    q, r = divmod(total, 512)
    if r == 0:
        return [512] * q
    if r >= 256:
        return [512] * q + [r]
    h1 = 256 + r // 2
    return [512] * (q - 1) + [h1, 512 + r - h1]


# ---------------------------------------------------------- device kernel IR
def _build_nc(routed_sizes):
    import concourse.bass as bass
    import concourse.tile as tile
    from concourse import bacc, mybir

    f32 = mybir.dt.float32
    bf16 = mybir.dt.bfloat16

    cap = sum(routed_sizes)
    # chunk schedule: (n, phase, x_off, o_off); offsets in elements
    chunks = []
    off = 0
    for n in SHARED_SIZES:
        chunks.append((n, "s", off))
        off += DIM * n
    for n in routed_sizes:
        chunks.append((n, "r", off))
        off += DIM * n
    tot_elems = off
    assert tot_elems == DIM * (NSH + cap)

    # weight layout in win (flat): name -> (kt, M, offset)
    wspecs = {}
    woff = 0
    for name, kt, M in [
        ("sw1", KD, INTER), ("sw3", KD, INTER), ("sw2", KI, DIM),
        ("w1", KD, INTER), ("w3", KD, INTER), ("w2", KI, DIM),
    ]:
        wspecs[name] = (kt, M, woff)
        woff += P * kt * M

    nc = bacc.Bacc(
        "TRN2",
        target_bir_lowering=False,
        debug=False,
        enable_asserts=False,
        num_devices=N_CORES,
    )

    xin = nc.dram_tensor("xin", [tot_elems], bf16, kind="ExternalInput").ap()
    win = nc.dram_tensor("win", [woff], bf16, kind="ExternalInput").ap()
    oout = nc.dram_tensor("oout", [tot_elems], bf16, kind="ExternalOutput").ap()

    with tile.TileContext(nc) as tc, ExitStack() as ctx:
        wpool = ctx.enter_context(tc.tile_pool(name="wpool", bufs=1))
        xpool = ctx.enter_context(tc.tile_pool(name="xpool", bufs=3))
        hpool = ctx.enter_context(tc.tile_pool(name="hpool", bufs=2))
        slpool = ctx.enter_context(tc.tile_pool(name="slpool", bufs=3))
        opool = ctx.enter_context(tc.tile_pool(name="opool", bufs=2))
        # PSUM banks: p1 x3 + p3 x2 + p2 x3 (warm shares p2) = 8 banks
        pp1 = ctx.enter_context(tc.tile_pool(name="pp1", bufs=3, space="PSUM"))
        pp3 = ctx.enter_context(tc.tile_pool(name="pp3", bufs=2, space="PSUM"))
        pp2 = ctx.enter_context(tc.tile_pool(name="pp2", bufs=3, space="PSUM"))

        # ---- x chunk loads: split across descriptors for DMA-engine parallelism
        xts = {}

        def issue_x(ci, nsplit, eng):
            n, _, xoff = chunks[ci]
            t = xpool.tile([P, KD, n], bf16, tag="xt", name=f"xt{ci}")
            per = P // nsplit
            blk = per * KD * n
            for s in range(nsplit):
                src = xin[xoff + s * blk : xoff + (s + 1) * blk].rearrange(
                    "(p k t) -> p k t", p=per, k=KD
                )
                eng.dma_start(out=t[s * per : (s + 1) * per], in_=src)
            xts[ci] = t

        def load_w(name, eng, nsplit):
            kt, M, woff_ = wspecs[name]
            t = wpool.tile([P, kt, M], bf16, tag=name, name=name)
            per = P // nsplit
            blk = per * kt * M
            for s in range(nsplit):
                src = win[woff_ + s * blk : woff_ + (s + 1) * blk].rearrange(
                    "(p k m) -> p k m", p=per, k=kt
                )
                eng.dma_start(out=t[s * per : (s + 1) * per], in_=src)
            return t

        # ---- prologue: first two x chunks + first-needed weights, multi-queue
        issue_x(0, 4, nc.sync)
        issue_x(1, 4, nc.sync)

        # PE warm-up: dummy matmuls on a zeroed tile while DMA is in flight
        warm = wpool.tile([P, 512], bf16, tag="warm", name="warm")
        nc.vector.memset(warm[:, :], 0.0)
        wps = pp2.tile([P, 512], f32, tag="p2", name="warmps")
        for _ in range(N_WARMUP):
            nc.tensor.matmul(wps[:, :], warm[:, 0:P], warm[:, :], start=True, stop=True)

        sw1s = load_w("sw1", nc.gpsimd, 4)
        sw3s = load_w("sw3", nc.scalar, 4)
        sw2s = load_w("sw2", nc.gpsimd, 4)
        wsets = {"s": (sw1s, sw3s, sw2s)}

        def swiglu_chunk(ci):
            n, phase, xoff = chunks[ci]
            a1, a3, a2 = wsets[phase]
            if ci + 2 < len(chunks):
                issue_x(ci + 2, 4, nc.sync)
            # stagger routed-weight loads during the early shared chunks
            if ci == 0:
                wsets.setdefault("r", [None, None, None])[0] = load_w("w1", nc.gpsimd, 4)
            elif ci == 1:
                wsets["r"][1] = load_w("w3", nc.gpsimd, 4)
            elif ci == 2:
                wsets["r"][2] = load_w("w2", nc.gpsimd, 4)

            xt = xts.pop(ci)
            h = hpool.tile([P, KI, n], bf16, tag="h", name=f"h{ci}")
            for m in range(KI):
                p1 = pp1.tile([P, n], f32, tag="p1", name="p1")
                for k in range(KD):
                    nc.tensor.matmul(
                        p1[:, :], a1[:, k, m * P : (m + 1) * P], xt[:, k, :],
                        start=(k == 0), stop=(k == KD - 1),
                    )
                sl = slpool.tile([P, n], bf16, tag="sl", name="sl")
                nc.scalar.activation(
                    sl[:, :], p1[:, :], mybir.ActivationFunctionType.Silu
                )
                p3 = pp3.tile([P, n], f32, tag="p3", name="p3")
                for k in range(KD):
                    nc.tensor.matmul(
                        p3[:, :], a3[:, k, m * P : (m + 1) * P], xt[:, k, :],
                        start=(k == 0), stop=(k == KD - 1),
                    )
                nc.vector.tensor_mul(h[:, m, :], sl[:, :], p3[:, :])
            ot = opool.tile([P, KD, n], bf16, tag="ot", name=f"ot{ci}")
            _, _, xo = chunks[ci]
            ooff = xo  # same layout/offsets as input
            for m2 in range(KD):
                p2 = pp2.tile([P, n], f32, tag="p2", name="p2")
                for k2 in range(KI):
                    nc.tensor.matmul(
                        p2[:, :], a2[:, k2, m2 * P : (m2 + 1) * P], h[:, k2, :],
                        start=(k2 == 0), stop=(k2 == KI - 1),
                    )
                nc.vector.tensor_copy(ot[:, m2, :], p2[:, :])
                if m2 % 2 == 1:
                    # drain pair (m2-1, m2): out block layout [pair][P][2][n]
                    j = m2 // 2
                    for q in range(4):
                        dst = oout[
                            ooff + j * (P * 2 * n) + q * 32 * (2 * n)
                            : ooff + j * (P * 2 * n) + (q + 1) * 32 * (2 * n)
                        ].rearrange("(p a t) -> p a t", p=32, a=2)
                        nc.gpsimd.dma_start(
                            out=dst, in_=ot[q * 32 : (q + 1) * 32, 2 * j : 2 * j + 2, :]
                        )

        for ci in range(len(chunks)):
            swiglu_chunk(ci)

    nc.compile()
    return nc, chunks


# -------------------------------------------------------------- host packing
def _blocks_from_cols(mat, sizes):
    """mat [DIM, ncols] bf16 -> flat concat of per-chunk [P, KD, n] blocks."""
    out = []
    o = 0
    for n in sizes:
        blk = np.ascontiguousarray(
            mat[:, o : o + n].reshape(KD, P, n).transpose(1, 0, 2)
        )
        out.append(blk.ravel())
        o += n
    return out


def _pack_weight(wt):
    """wt [out, in] f32 -> lhsT tile layout [P, kt, out] bf16, flattened."""
    kt = wt.shape[1] // P
    wT = wt.T.astype(BF)  # [in, out]
    return np.ascontiguousarray(
        wT.reshape(kt, P, wt.shape[0]).transpose(1, 0, 2)
    ).ravel()


def _decode_chunk(seg, n):
    """flat bf16 chunk block [pair][P][2][n] -> [DIM, n] fp32."""
    return (
        seg.reshape(3, P, 2, n).transpose(0, 2, 1, 3).reshape(DIM, n)
        .astype(np.float32)
    )


# ------------------------------------------------------------------- driver
def kernel(x, gate_weight, gate_bias, w1, w2, w3, sw1, sw2, sw3):
    from concourse.bass_utils import run_bass_kernel_spmd

    B, S, D = x.shape
    x2 = np.ascontiguousarray(x.reshape(-1, D))
    T = x2.shape[0]
    assert T == N_CORES * NSH

    w, idx = _host_gate(x2, gate_weight, gate_bias)

    rows_per_e = [np.nonzero((idx == e).any(axis=1))[0] for e in range(E)]
    cap = max(len(r) for r in rows_per_e)
    cap = ((cap + 63) // 64) * 64
    routed_sizes = _plan(cap)

    nc, chunks = _build_nc(routed_sizes)

    x2T = np.ascontiguousarray(x2.T).astype(BF)  # [D, T]

    wflat = np.concatenate(
        [_pack_weight(sw1), _pack_weight(sw3), _pack_weight(sw2),
         _pack_weight(np.zeros((INTER, DIM), np.float32)),  # placeholder w1
         _pack_weight(np.zeros((INTER, DIM), np.float32)),  # placeholder w3
         _pack_weight(np.zeros((DIM, INTER), np.float32))]  # placeholder w2
    )
    w_sz = {"w1": P * KD * INTER, "w3": P * KD * INTER, "w2": P * KI * DIM}

    # weight segment offsets in wflat (must match _build_nc order)
    offs = {}
    o = 0
    for name, kt, M in [("sw1", KD, INTER), ("sw3", KD, INTER), ("sw2", KI, DIM),
                        ("w1", KD, INTER), ("w3", KD, INTER), ("w2", KI, DIM)]:
        offs[name] = o
        o += P * kt * M

    in_maps = []
    for e in range(E):
        rows = rows_per_e[e]
        xg = np.zeros((DIM, cap), BF)
        xg[:, : len(rows)] = x2T[:, rows]
        xs = x2T[:, e * NSH : (e + 1) * NSH]
        xin = np.concatenate(
            _blocks_from_cols(xs, SHARED_SIZES) + _blocks_from_cols(xg, routed_sizes)
        )
        wf = wflat.copy()
        wf[offs["w1"] : offs["w1"] + w_sz["w1"]] = _pack_weight(w1[e])
        wf[offs["w3"] : offs["w3"] + w_sz["w3"]] = _pack_weight(w3[e])
        wf[offs["w2"] : offs["w2"] + w_sz["w2"]] = _pack_weight(w2[e])
        in_maps.append({"xin": xin, "win": wf})

    r = run_bass_kernel_spmd(nc, in_maps, list(range(N_CORES)))
    globals()["LAST_RESULTS"] = r
    res = r.results

    y = np.zeros((T, D), np.float32)
    z = np.empty((T, D), np.float32)
    for e in range(E):
        rows = rows_per_e[e]
        arr = np.asarray(res[e]["oout"]).view(BF).ravel()
        # shared chunks
        o = 0
        tok = e * NSH
        for n in SHARED_SIZES:
            z[tok : tok + n] = _decode_chunk(arr[o : o + DIM * n], n).T
            tok += n
            o += DIM * n
        # routed chunks
        col = 0
        we = np.where(idx[rows, 0] == e, w[rows, 0], w[rows, 1]).astype(np.float32)
        for n in routed_sizes:
            blk = _decode_chunk(arr[o : o + DIM * n], n)  # [D, n]
            lo, hi = col, min(col + n, len(rows))
            if lo < hi:
                rr = rows[lo:hi]
                y[rr] += we[lo:hi, None] * blk[:, : hi - lo].T
            col += n
            o += DIM * n
    return (y + z).reshape(B, S, D)
